# revision 1
# baseline (speedup 1.0000x reference)
"""Trainium2 Bass kernel for nn_MultiHeadAttention_66984309948505.

Full causal MHA: x[4,2048,1024], 16 heads of 64, out-proj + bias.

Sharding (8 cores): 4-way data-parallel over batch x 2-way tensor-parallel
over heads. Core (b, g) computes heads [8g, 8g+8) for batch b, including the
partial output projection Y_partial = O_g @ Wo[:, 512g:512(g+1)].T.
Host-side unshard: Y[b] = (Y_partial[b,g=0] + Y_partial[b,g=1]).T + bo.

Device layouts are all "transposed" (feature-major) so no on-chip transposes
are needed anywhere:
  xt  [c, p, dt, s] p-major tiling of x[b].T    (host pre-transpose)
  QT/KT [S, T]  with S = 8 heads * 64, head h at partitions (h%2)*64 of
                 stack h//2 (stacks of 128 partitions)
  scores S^T[k, q] per head-pair (two heads packed in PE row groups);
                 softmax denominator comes free from a ones-column
                 appended to V in the AV matmul (PSUM row 64)
  O^T [S, T], Y^T [D, T] -> host transposes back.

v2 scheduling: all matmul operands bf16 (fp32 PSUM accumulate). The
per-chunk work is emitted as a single interleaved stream: each kt step of
the attention inner loop emits scores(kt), exp(kt), a few "filler" matmuls
(QKV projections for the next chunk + the previous chunk's output
projection), then AV(kt-2) -- a depth-2 software pipeline so the PE never
stalls on the exp latency and the projections ride in the PE bubbles that
the Activation engine would otherwise create. Softmax normalization uses
reciprocal (DVE) + partition_broadcast (GPSIMD, otherwise idle) + one
fused multiply (DVE).
"""

import math
from collections import deque

import numpy as np
import ml_dtypes

import concourse.bacc as bacc
import concourse.bass as bass
import concourse.mybir as mybir
import concourse.tile as tile
from concourse.bass_utils import run_bass_kernel_spmd

# Problem constants (hardcoded per contract)
B, T, D = 4, 2048, 1024
H, HS = 16, 64
NCORES = 8
HG = 2                 # head-group TP degree
H_LOC = H // HG        # 8 heads per core
S = H_LOC * HS         # 512 local head dims
P = 128
TCH = 512              # t/q chunk width
NCHUNK = T // TCH      # 4
ND = D // P            # 8 d-tiles
NSP = S // P           # 4 head stacks
NTT = TCH // P         # 4 k-subtiles per chunk
SCALE = 1.0 / math.sqrt(HS)

F32 = mybir.dt.float32
BF16 = mybir.dt.bfloat16
F8 = mybir.dt.float8e4
EXP = mybir.ActivationFunctionType.Exp
DR = mybir.MatmulPerfMode.DoubleRow


def build_program(reps: int = 1, mmdt=BF16):
    nc = bacc.Bacc("TRN2", target_bir_lowering=False, debug=False)

    xt = nc.dram_tensor("xt", [NCHUNK, P, ND, TCH], mmdt, kind="ExternalInput")
    wq = nc.dram_tensor("wq", [P, ND, S], mmdt, kind="ExternalInput")
    wk = nc.dram_tensor("wk", [P, ND, S], mmdt, kind="ExternalInput")
    # stack-0 weight columns as separate contiguous tensors: the strided
    # [:, :, 0:128] slices would pay the 2x small-element DMA penalty on
    # the startup critical path
    wqh = nc.dram_tensor("wqh", [P, ND, P], mmdt, kind="ExternalInput")
    wkh = nc.dram_tensor("wkh", [P, ND, P], mmdt, kind="ExternalInput")
    wv = nc.dram_tensor("wv", [P, ND, S], mmdt, kind="ExternalInput")
    wot = nc.dram_tensor("wot", [P, NSP, D], mmdt, kind="ExternalInput")
    tri2 = nc.dram_tensor("tri2", [P, 2, P], mmdt, kind="ExternalInput")
    # scratch for the Q^T/K^T fp8 repack roundtrip (partition-split views
    # are only expressible on the DRAM side of a DMA)
    qk8scr = nc.dram_tensor("qk8scr", [NCHUNK, 2, NSP, P, TCH], F8,
                            kind="Internal")
    # partition-major output: yt[p, et, t] holds Y^T[et*128 + p, t].
    # bf16: the host sums the two tensor-parallel partials in f32; the
    # rounding here adds ~0.3% error against a 2% tolerance.
    yt = nc.dram_tensor("yt", [P, ND, T], mmdt, kind="ExternalOutput")

    with tile.TileContext(nc) as tc:
        with (
            nc.allow_low_precision(reason="bf16 matmul operands, fp32 accum"),
            tc.tile_pool(name="const", bufs=1) as constp,
            tc.tile_pool(name="kv", bufs=1) as kvp,
            tc.tile_pool(name="qt", bufs=2) as qtp,
            tc.tile_pool(name="osb", bufs=4) as osbp,
            tc.tile_pool(name="osb2", bufs=4) as osbp2,
            tc.tile_pool(name="xp", bufs=2) as xp,
            tc.tile_pool(name="ptp", bufs=3) as ptp,
            tc.tile_pool(name="rcpp", bufs=2) as rcpp,
            tc.tile_pool(name="bcp", bufs=2) as bcp,
            tc.tile_pool(name="stg", bufs=4) as stp,
            tc.tile_pool(name="f8p", bufs=2) as f8p,
            tc.tile_pool(name="orp", bufs=4) as orp,
            tc.tile_pool(name="psS", bufs=2, space="PSUM") as psS,
            tc.tile_pool(name="psO", bufs=2, space="PSUM") as psO,
            tc.tile_pool(name="psF", bufs=2, space="PSUM") as psF,
        ):
            # Constants / resident weights. Single consolidated transfers
            # (contiguous on both sides) -- the sync queue issues one DMA
            # per ~650ns, so transfer count, not bytes, gates the head.
            tri_sb = constp.tile([P, 2, P], mmdt, name="tri_sb")
            wq_sb = constp.tile([P, ND, S], mmdt, name="wq_sb")
            wk_sb = constp.tile([P, ND, S], mmdt, name="wk_sb")
            wqh_sb = constp.tile([P, ND, P], mmdt, name="wqh_sb")
            wkh_sb = constp.tile([P, ND, P], mmdt, name="wkh_sb")
            wv_sb = constp.tile([P, ND, S], mmdt, name="wv_sb")
            wot_sb = constp.tile([P, NSP, D], mmdt, name="wot_sb")
            xt_tiles = {}

            def emit_x_dma(c):
                if c not in xt_tiles:
                    xt_tiles[c] = xp.tile([P, ND, TCH], mmdt, tag="x",
                                          name=f"x{c}")
                    nc.sync.dma_start(out=xt_tiles[c][:], in_=xt[c])

            if reps > 1:
                nc.sync.dma_start(out=wq_sb[:], in_=wq[:])
                nc.sync.dma_start(out=wk_sb[:], in_=wk[:])
                nc.sync.dma_start(out=wqh_sb[:], in_=wqh[:])
                nc.sync.dma_start(out=wkh_sb[:], in_=wkh[:])
                nc.sync.dma_start(out=tri_sb[:], in_=tri2[:])
                nc.sync.dma_start(out=wv_sb[:], in_=wv[:])
                nc.sync.dma_start(out=wot_sb[:], in_=wot[:])

            # Resident K^T (fp8, DoubleRow-packed [32, st, 4, T]) and V
            # (bf16) -- per-chunk tiles for clean dep tracking.
            # Chunk 0 also keeps bf16 Q^T/K^T: its own scores use them (the
            # ~2.5us fp8 repack latency would sit on the startup critical
            # path, and c0's score volume is tiny), while the fp8 K(0)
            # repack still happens for chunks 1-3.
            kt_sb = [kvp.tile([32, NSP, 4, TCH], F8, name=f"kt{c}")
                     for c in range(NCHUNK)]
            qt_bf0 = constp.tile([P, NSP, TCH], mmdt, name="qt_bf0")
            kt_bf0 = constp.tile([P, NSP, TCH], mmdt, name="kt_bf0")
            v_sb = [kvp.tile([P, NTT, H_LOC, HS + 1], mmdt, name=f"v{c}")
                    for c in range(NCHUNK)]
            for c in range(NCHUNK):
                nc.any.memset(v_sb[c][:, :, :, HS:HS + 1], 1.0)

            qt_tiles = {}
            o_sb_tiles = {}
            # filler units mark completion here; the attention stream pulls
            # fillers forward when it reaches a consumer whose producer has
            # not been emitted yet (deps only bind to already-emitted writes)
            unit_done = set()

            def emit_qt_stack(c, which, st):
                """One PSUM accumulation group: Q^T (which=0) or K^T
                (which=1) head-stack st for chunk c, cast to fp8 and
                repacked to the DoubleRow layout [32, 4, T] (dest[p, b, t] =
                stack-row[32b + p, t]) via a DRAM roundtrip. Yields after
                each matmul."""
                if c > 0 and c not in qt_tiles:
                    qt_tiles[c] = qtp.tile([32, NSP, 4, TCH], F8, tag="qt",
                                           name=f"qt{c}")
                if st == 0:
                    w_sb, wsl = (wqh_sb, wkh_sb)[which], slice(0, P)
                else:
                    w_sb = (wq_sb, wk_sb)[which]
                    wsl = slice(st * P, (st + 1) * P)
                ps = psF.tile([P, TCH], F32, tag="fill", name="ps_f")
                for dt in range(ND):
                    nc.tensor.matmul(
                        ps[:], w_sb[:, dt, wsl],
                        xt_tiles[c][:, dt, :],
                        start=(dt == 0), stop=(dt == ND - 1))
                    if dt == ND - 1:
                        if c == 0:
                            # Activation engine is lightly loaded in chunk 0;
                            # putting these copies there unblocks the PE from
                            # the serial DVE copy chain
                            dst_bf = (qt_bf0, kt_bf0)[which]
                            nc.scalar.activation(
                                dst_bf[:, st, :], ps[:],
                                mybir.ActivationFunctionType.Copy)
                        if c > 0 or which == 1:
                            tmp = f8p.tile([P, TCH], F8, tag="f8",
                                           name="f8tmp")
                            nc.vector.tensor_copy(tmp[:], ps[:])
                            scr = qk8scr[c, which, st]
                            nc.sync.dma_start(out=scr, in_=tmp[:])
                            dst = (qt_tiles.get(c), kt_sb[c])[which]
                            nc.sync.dma_start(
                                out=dst[:, st, :, :],
                                in_=scr.rearrange("(b p) n -> p b n", p=32))
                        unit_done.add(("qk"[which], c, st))
                    yield

            def emit_v_tt(c, tt, s0=0, s1=S):
                """One PSUM group: V k-subtile tt (heads covering s-dims
                [s0, s1)) for chunk c."""
                nh = (s1 - s0) // HS
                ps = psF.tile([P, TCH], F32, tag="fill", name="ps_f")
                for dt in range(ND):
                    nc.tensor.matmul(
                        ps[:, s0:s1], xt_tiles[c][:, dt, tt * P:(tt + 1) * P],
                        wv_sb[:, dt, s0:s1],
                        start=(dt == 0), stop=(dt == ND - 1))
                    if dt == ND - 1:
                        nc.vector.tensor_copy(
                            v_sb[c][:, tt, s0 // HS:s1 // HS, 0:HS],
                            ps[:, s0:s1].rearrange("p (h e) -> p h e", h=nh))
                        if s0 == 0:
                            unit_done.add(("vlo", c, tt))
                        if s1 == S:
                            unit_done.add(("vhi", c, tt))
                    yield

            def osb(c, sp):
                """o_sb is split in two tiles (stacks 0:3 | stack 3) so the
                final out-projection's early partial accumulations don't
                falsely depend on the last head-pair's normalization."""
                a, b = o_sb_tiles[c]
                return (a, sp) if sp < NSP - 1 else (b, 0)

            def emit_op_et(c, et):
                """One PSUM group: output-projection d-tile et for chunk c."""
                ps = psF.tile([P, TCH], F32, tag="fill", name="ps_f")
                for sp in range(NSP):
                    t, i = osb(c, sp)
                    nc.tensor.matmul(
                        ps[:], wot_sb[:, sp, et * P:(et + 1) * P],
                        t[:, i, :],
                        start=(sp == 0), stop=(sp == NSP - 1))
                    if sp == NSP - 1:
                        st_t = stp.tile([P, TCH], mmdt, tag="y", name="st_t")
                        nc.vector.tensor_copy(st_t[:], ps[:])
                        nc.sync.dma_start(
                            out=yt[:, et, c * TCH:(c + 1) * TCH],
                            in_=st_t[:])
                    yield

            def gen_proj(c, order):
                """Projection fillers for chunk c in the given unit order."""
                emit_x_dma(c)
                for kind, idx in order:
                    if kind == "q":
                        yield from emit_qt_stack(c, 0, idx)
                    elif kind == "k":
                        yield from emit_qt_stack(c, 1, idx)
                    elif kind == "vh":  # head-pair 0's slice of V only
                        yield from emit_v_tt(c, idx, 0, P)
                    elif kind == "vr":  # the rest of V
                        yield from emit_v_tt(c, idx, P, S)
                    else:
                        yield from emit_v_tt(c, idx)

            def gen_op(c):
                for et in range(ND):
                    yield from emit_op_et(c, et)

            # chunk-0 attention starts after Q/K stack 0; the rest arrives as
            # filler, ordered so each consumer finds its tile ready:
            # head-pair-0's V tt0 slice (AV kt0), Q/K stack 1 (hp1), rest.
            C0_ORDER = [("vh", 0), ("q", 1), ("k", 1), ("vr", 0), ("v", 1),
                        ("v", 2), ("v", 3), ("q", 2), ("k", 2), ("q", 3),
                        ("k", 3)]
            # interleave q/k per stack so hp groups unblock in order
            PROJ_QK = []
            for st in range(NSP):
                PROJ_QK += [("q", st), ("k", st)]
            V_ORDER = [("v", tt) for tt in range(NTT)]

            deferred = []

            def emit_head():
                """Chunk-0 head with DMA emissions interleaved between the
                first matmuls: dependencies are tracked against previously
                EMITTED writes only, so each QT0 matmul waits just for the
                wq stack-0 / x0 quarter it actually reads."""
                xt_tiles[0] = xp.tile([P, ND, TCH], mmdt, tag="x", name="x0")
                nc.sync.dma_start(out=wqh_sb[:, 0:4], in_=wqh[:, 0:4])
                nc.sync.dma_start(out=xt_tiles[0][:, 0:2], in_=xt[0, :, 0:2])
                ps = psF.tile([P, TCH], F32, tag="fill", name="ps_f")
                for dt in range(ND):
                    nc.tensor.matmul(
                        ps[:], wqh_sb[:, dt, :], xt_tiles[0][:, dt, :],
                        start=(dt == 0), stop=(dt == ND - 1))
                    if dt == 1:
                        nc.sync.dma_start(out=xt_tiles[0][:, 2:4],
                                          in_=xt[0, :, 2:4])
                        nc.sync.dma_start(out=wqh_sb[:, 4:], in_=wqh[:, 4:])
                    elif dt == 3:
                        nc.sync.dma_start(out=wkh_sb[:], in_=wkh[:])
                        nc.sync.dma_start(out=xt_tiles[0][:, 4:6],
                                          in_=xt[0, :, 4:6])
                    elif dt == 5:
                        nc.sync.dma_start(out=xt_tiles[0][:, 6:8],
                                          in_=xt[0, :, 6:8])

                nc.vector.tensor_copy(qt_bf0[:, 0, :], ps[:])
                unit_done.add(("q", 0, 0))
                ps = psF.tile([P, TCH], F32, tag="fill", name="ps_f")
                for dt in range(ND):
                    nc.tensor.matmul(
                        ps[:], wkh_sb[:, dt, :], xt_tiles[0][:, dt, :],
                        start=(dt == 0), stop=(dt == ND - 1))
                    if dt == 0:
                        nc.sync.dma_start(out=wv_sb[:, :, 0:P],
                                          in_=wv[:, :, 0:P])
                    elif dt == 2:
                        nc.sync.dma_start(out=tri_sb[:], in_=tri2[:])
                nc.vector.tensor_copy(kt_bf0[:, 0, :], ps[:])
                # fp8 repack of K(0) stack 0 for chunks 1-3 (latency hidden)
                tmp = f8p.tile([P, TCH], F8, tag="f8", name="f8tmp")
                nc.vector.tensor_copy(tmp[:], ps[:])
                nc.sync.dma_start(out=qk8scr[0, 1, 0], in_=tmp[:])
                nc.sync.dma_start(
                    out=kt_sb[0][:, 0, :, :],
                    in_=qk8scr[0, 1, 0].rearrange("(b p) n -> p b n", p=32))
                unit_done.add(("k", 0, 0))
                # bulk of the weights, off the startup critical path
                nc.sync.dma_start(out=wq_sb[:, :, P:], in_=wq[:, :, P:])
                nc.sync.dma_start(out=wk_sb[:, :, P:], in_=wk[:, :, P:])
                nc.sync.dma_start(out=wv_sb[:, :, P:], in_=wv[:, :, P:])

            def emit_body():
                # head: x0 + Q/K stack 0 of chunk 0
                if reps == 1:
                    emit_head()
                else:
                    for _ in gen_proj(0, [("q", 0), ("k", 0)]):
                        pass

                # Filler assignment per chunk, balanced so every chunk's
                # steps carry >=1.9 filler matmuls (the Activation engine
                # needs ~1038ns/step vs ~640ns of fp8 attention PE work).
                # Each chunk produces its own V (consumed only at its
                # diagonal steps; `ensure` guards the ordering), the next
                # chunk's Q/K, and the deferred output projections.
                chunk_fillers = {
                    0: [(gen_proj(0, C0_ORDER), len(C0_ORDER) * ND),
                        (gen_proj(1, PROJ_QK), len(PROJ_QK) * ND)],
                    1: [(gen_proj(1, V_ORDER), len(V_ORDER) * ND),
                        (gen_proj(2, PROJ_QK), len(PROJ_QK) * ND)],
                    2: [(gen_proj(2, V_ORDER), len(V_ORDER) * ND),
                        (gen_proj(3, PROJ_QK), len(PROJ_QK) * ND)],
                    3: [(gen_proj(3, V_ORDER), len(V_ORDER) * ND),
                        (gen_op(0), ND * NSP), (gen_op(1), ND * NSP),
                        (gen_op(2), ND * NSP)],
                }
                for c in range(NCHUNK):
                    if c == 1 and reps == 1:
                        # wot is first needed by outproj fillers in chunk 2+;
                        # emitting it here keeps chunk 0's DMA queue clear
                        # for the Q/K fp8 repacks
                        nc.sync.dma_start(out=wot_sb[:], in_=wot[:])
                    o_sb_tiles[c] = (
                        osbp.tile([P, NSP - 1, TCH], mmdt, tag="o",
                                  name=f"o{c}a"),
                        osbp2.tile([P, 1, TCH], mmdt, tag="ob",
                                   name=f"o{c}b"))
                    fillers = deque(g for g, _ in chunk_fillers[c])
                    n_fill = sum(n for _, n in chunk_fillers[c])
                    # weight the filler pops by each step's exp width so PE
                    # cover concentrates where the Activation engine is
                    # slowest (full-width steps)
                    wsum = 4 * ((4 * c + 1) + 1.5) + 2
                    state = {"carry": 0.0, "rate": n_fill / wsum}

                    def pop_fillers(weight=1.0):
                        state["carry"] += state["rate"] * weight
                        n = int(state["carry"])
                        state["carry"] -= n
                        while n > 0 and fillers:
                            try:
                                next(fillers[0])
                                n -= 1
                            except StopIteration:
                                fillers.popleft()

                    def drain_fillers():
                        while fillers:
                            try:
                                next(fillers[0])
                            except StopIteration:
                                fillers.popleft()

                    def ensure(key):
                        while key not in unit_done and fillers:
                            try:
                                next(fillers[0])
                            except StopIteration:
                                fillers.popleft()
                        assert key in unit_done, f"missing producer {key}"

                    emit_chunk(c, pop_fillers, drain_fillers, ensure)

                    while deferred:
                        deferred.pop(0)()
                # Final output projection. The first two d-tiles accumulate
                # their first 3 head-stacks early (those norms landed long
                # ago) so the PE works while the last head-pair's direct
                # normalization chain finishes.
                cl = NCHUNK - 1
                NPART = 6  # et 0..5 accumulate sp 0..2 during the norm chain
                part = {}
                for et in range(NPART):
                    # the attention-score and o-accumulator PSUM pools are
                    # idle (or freeing) by now; borrow their buffers for
                    # four extra partial accumulators
                    pool = (psF, psF, psS, psS, psO, psO)[et]
                    ps = pool.tile([P, TCH], F32,
                                   tag=("fill", "fill", "s", "s", "o_ps",
                                        "o_ps")[et],
                                   name="ps_part")
                    part[et] = ps
                    for sp in range(NSP - 1):
                        t, i = osb(cl, sp)
                        nc.tensor.matmul(
                            ps[:], wot_sb[:, sp, et * P:(et + 1) * P],
                            t[:, i, :],
                            start=(sp == 0), stop=False)

                # Stores: copies alternate between DVE and the now-idle
                # Activation engine; DMAs go out in et-PAIRS (one transfer
                # per pair) since the sync queue's serial ~650ns issue cost
                # per DMA would otherwise pace the tail.
                st2 = {}
                cp_tog = [0]

                def store_y(et, ps, sl=slice(0, TCH)):
                    pr, half = et // 2, et % 2
                    if pr not in st2:
                        st2[pr] = stp.tile([P, 2, TCH], mmdt, tag="y",
                                           name="st_t")
                    if cp_tog[0] == 0:
                        nc.vector.tensor_copy(st2[pr][:, half, sl], ps[:, sl])
                    else:
                        nc.scalar.activation(
                            st2[pr][:, half, sl], ps[:, sl],
                            mybir.ActivationFunctionType.Copy)
                    cp_tog[0] ^= 1
                    if half == 1:
                        nc.sync.dma_start(
                            out=yt[:, 2 * pr:2 * pr + 2,
                                   cl * TCH + sl.start:cl * TCH + sl.stop],
                            in_=st2[pr][:, :, sl])

                for et in range(NPART):
                    t, i = osb(cl, NSP - 1)
                    nc.tensor.matmul(
                        part[et][:], wot_sb[:, NSP - 1, et * P:(et + 1) * P],
                        t[:, i, :],
                        start=False, stop=True)
                    store_y(et, part[et])
                for et in range(NPART, ND):
                    ps = psF.tile([P, TCH], F32, tag="fill", name="ps_f")
                    for sp in range(NSP):
                        t, i = osb(cl, sp)
                        nc.tensor.matmul(
                            ps[:], wot_sb[:, sp, et * P:(et + 1) * P],
                            t[:, i, :],
                            start=(sp == 0), stop=(sp == NSP - 1))
                    if et < ND - 1:
                        store_y(et, ps)
                    else:
                        # split the last pair's store in column halves so
                        # the final DMA overlaps the final copy
                        store_y(et, ps, slice(0, TCH // 2))
                        store_y(et, ps, slice(TCH // 2, TCH))

            def emit_chunk(c, pop_fillers, drain_fillers, ensure):
                """One continuous software-pipelined stream over all
                (head-pair, kt) steps of the chunk: AV trails the scores by
                two steps, including across head-pair boundaries, so the PE
                never drains at a seam."""
                qt_c = qt_tiles.get(c)
                nkt = 4 * c + 4
                steps = [(hp, kt) for hp in range(H_LOC // 2)
                         for kt in range(nkt)]
                o_ps_hp = {}
                pts = {}

                def emit_norm(hp):
                    if c == NCHUNK - 1 and hp == H_LOC // 2 - 1:
                        # last head-pair of the kernel: nothing left to hide
                        # the deferral behind -- emit the whole chain now,
                        # straight out of PSUM (skip the o_raw staging copy)
                        for j in range(2):
                            rcp = rcpp.tile([P, TCH], mmdt, tag="rcp",
                                            name="rcp")
                            nc.vector.reciprocal(rcp[0:1, :],
                                                 o_ps_hp[hp][j][64:65, :])
                            bc = bcp.tile([P, TCH], mmdt, tag="bc", name="bc")
                            nc.gpsimd.partition_broadcast(
                                bc[0:64, :], rcp[0:1, :], channels=64)
                            t, i = osb(c, hp)
                            nc.vector.tensor_mul(
                                t[j * 64:(j + 1) * 64, i, :],
                                o_ps_hp[hp][j][0:64, :], bc[0:64, :])
                        del o_ps_hp[hp]
                        return
                    # Normalization, phase 1: a single 65-row copy (o rows
                    # 0:64 + ones-column rowsum at row 64) frees the PSUM
                    # accumulator fast. Phase 2 (reciprocal ->
                    # partition_broadcast on the idle GPSIMD -> all-bf16
                    # multiply into o_sb) is deferred off the critical path.
                    for j in range(2):
                        o_raw = orp.tile([P, TCH], mmdt, tag="oraw",
                                         name="o_raw")
                        nc.vector.tensor_copy(o_raw[0:65, :],
                                              o_ps_hp[hp][j][0:65, :])

                        def norm(j=j, o_raw=o_raw, c=c, hp=hp):
                            rcp = rcpp.tile([P, TCH], mmdt, tag="rcp",
                                            name="rcp")
                            nc.vector.reciprocal(rcp[0:1, :], o_raw[64:65, :])
                            bc = bcp.tile([P, TCH], mmdt, tag="bc", name="bc")
                            nc.gpsimd.partition_broadcast(
                                bc[0:64, :], rcp[0:1, :], channels=64)

                            def mul():
                                t, i = osb(c, hp)
                                nc.vector.tensor_mul(
                                    t[j * 64:(j + 1) * 64, i, :],
                                    o_raw[0:64, :], bc[0:64, :])
                            deferred.append(mul)
                        deferred.append(norm)
                    del o_ps_hp[hp]

                def emit_av(hp, kt):
                    if hp not in o_ps_hp:
                        o_ps_hp[hp] = [psO.tile([P, TCH], F32, tag="o_ps",
                                                name="o_ps")
                                       for _ in range(2)]
                    cc, tt = kt // 4, kt % 4
                    ensure(("vlo", cc, tt))
                    if hp > 0:
                        ensure(("vhi", cc, tt))
                    q0 = max(0, P * kt - TCH * c)
                    for j in range(2):
                        h = 2 * hp + j
                        nc.tensor.matmul(
                            o_ps_hp[hp][j][0:HS + 1, q0:],
                            v_sb[cc][:, tt, h, :],
                            pts[(hp, kt)][:, j, q0:],
                            start=(kt == 0), stop=(kt == nkt - 1))
                    del pts[(hp, kt)]
                    if kt == nkt - 1:
                        emit_norm(hp)

                for idx, (hp, kt) in enumerate(steps):
                    cc, tt = kt // 4, kt % 4
                    q0 = max(0, P * kt - TCH * c)
                    ensure(("q", c, hp))
                    ensure(("k", cc, hp))
                    s_ps = psS.tile([P, 2, TCH], F32, tag="s", name="s_ps")
                    for j in range(2):
                        if c == 0:  # bf16 path (no repack latency at start)
                            rows = slice(j * 64, j * 64 + 64)
                            nc.tensor.matmul(
                                s_ps[:, j, q0:],
                                kt_bf0[rows, hp, tt * P:(tt + 1) * P],
                                qt_bf0[rows, hp, q0:],
                                start=True, stop=True)
                            continue
                        # fp8 DoubleRow: head j's 64 dims live in blocks
                        # 2j:2j+2 as (p, b) pairs; 0.5 cycles/column
                        nc.tensor.matmul(
                            s_ps[:, j, q0:],
                            kt_sb[cc][:, hp, 2 * j:2 * j + 2,
                                      tt * P:(tt + 1) * P],
                            qt_c[:, hp, 2 * j:2 * j + 2, q0:],
                            start=True, stop=True, perf_mode=DR)
                    pt = ptp.tile([P, 2, TCH], BF16, tag="pt", name="pt")
                    pts[(hp, kt)] = pt
                    nc.scalar.activation(
                        pt[:, :, q0:], s_ps[:, :, q0:], EXP, scale=float(SCALE))
                    if kt >= 4 * c:  # diagonal block: causal tri mask
                        nc.vector.tensor_mul(
                            pt[:, :, q0:q0 + P], pt[:, :, q0:q0 + P],
                            tri_sb[:])
                    pop_fillers((TCH - q0) / TCH)
                    if deferred:
                        deferred.pop(0)()
                    if idx >= 2:
                        emit_av(*steps[idx - 2])
                drain_fillers()
                emit_av(*steps[-2])
                emit_av(*steps[-1])

            import contextlib
            loop_ctx = tc.For_i(0, reps, 1) if reps > 1 else contextlib.nullcontext()
            with loop_ctx:
                emit_body()

    nc.compile()
    return nc


_CACHE = {}


def _get_program(reps: int = 1, mmdt=BF16):
    key = ("nc", reps, str(mmdt))
    if key not in _CACHE:
        _CACHE[key] = build_program(reps, mmdt)
    return _CACHE[key]


def make_in_maps(x, Wq, Wk, Wv, Wo, npdt=ml_dtypes.bfloat16):
    x = np.asarray(x, dtype=np.float32)
    Wq = np.asarray(Wq, dtype=np.float32)
    Wk = np.asarray(Wk, dtype=np.float32)
    Wv = np.asarray(Wv, dtype=np.float32)
    Wo = np.asarray(Wo, dtype=np.float32)
    tri = np.triu(np.ones((P, P), dtype=np.float32))
    tri2 = np.ascontiguousarray(
        np.broadcast_to(tri[:, None, :], (P, 2, P)))

    def wmat(W, g):
        # [H_LOC, D, HS] -> [D, S] (s = h_local*HS + e) -> p-major [P, ND, S]
        m = W[g * H_LOC:(g + 1) * H_LOC].transpose(1, 0, 2).reshape(D, S)
        return np.ascontiguousarray(m.reshape(ND, P, S).transpose(1, 0, 2))

    in_maps = []
    for core in range(NCORES):
        b, g = core // HG, core % HG
        xT = x[b].T  # [D, T]
        xt_t = np.ascontiguousarray(
            xT.reshape(ND, P, NCHUNK, TCH).transpose(2, 1, 0, 3))
        woT = Wo[:, g * S:(g + 1) * S].T  # [S, D]
        wot_t = np.ascontiguousarray(woT.reshape(NSP, P, D).transpose(1, 0, 2))
        wq_m = wmat(Wq, g).astype(npdt)
        wk_m = wmat(Wk, g).astype(npdt)
        in_maps.append({
            "xt": xt_t.astype(npdt),
            "wq": wq_m,
            "wk": wk_m,
            "wqh": np.ascontiguousarray(wq_m[:, :, 0:P]),
            "wkh": np.ascontiguousarray(wk_m[:, :, 0:P]),
            "wv": wmat(Wv, g).astype(npdt),
            "wot": wot_t.astype(npdt),
            "tri2": tri2.astype(npdt),
        })
    return in_maps


def kernel_ex(x, Wq, Wk, Wv, Wo, bo, **run_kwargs):
    """Run and return (output, BassKernelResults)."""
    nc = _get_program()
    in_maps = make_in_maps(x, Wq, Wk, Wv, Wo)
    res = run_bass_kernel_spmd(nc, in_maps, core_ids=list(range(NCORES)),
                               **run_kwargs)
    bo = np.asarray(bo, dtype=np.float32)
    y = np.empty((B, T, D), dtype=np.float32)
    for b in range(B):
        # yt is [P, ND, T] partition-major; Y^T[et*128 + p, t] = yt[p, et, t]
        ytf = (res.results[HG * b]["yt"].astype(np.float32)
               + res.results[HG * b + 1]["yt"].astype(np.float32))
        y[b] = ytf.transpose(1, 0, 2).reshape(D, T).T + bo
    return y, res


def kernel(x, Wq, Wk, Wv, Wo, bo):
    y, _ = kernel_ex(x, Wq, Wk, Wv, Wo, bo)
    return y

